# revision 42
# baseline (speedup 1.0000x reference)
"""Trainium2 Bass kernel for the windowed 3-channel MLP (dense_mlp).

Reference computation (B=8192):
  x [B, 6144] -> view [B, 3, 2048]
  16 overlapping windows/channel (len 256, stride 119)
  h[b,c,w,:] = win @ W1[c,w] + b1[c,w]          # [B,3,16,64]
  h = mean over c                               # [B,16,64]
  g[b,grp]   = h-grp(4 windows=256) @ W2[grp] + b2   # [B,4,64]
  out        = g.reshape(B,256) @ W3 + b3       # [B,255]

Strategy: pure data parallelism over 8 cores (B/8 = 1024 rows each).
x is cast fp16, pre-transposed and chunk-packed on the HOST into the
exact feature-major SBUF layout the kernel consumes (plain contiguous
HBM->SBUF DMAs). W1 is stored fp8 e3m4 (scaled x64 into its dense
range; 1/64 folded into W2): fp8 weights halve the LDWEIGHTS stream
(4-byte FWL), and LDWEIGHTS is ~70% serialized with the matmuls on
this codegen path, so this bought ~9us. f32 PSUM accumulate; fp16
between layers. End-to-end rel err 1.35e-2 (gate 2e-2).

On-device per core:
  - 2 batch superchunks of 512 (maximizes matmul free dim = fewest
    matmul instructions; PSUM bank limit is 512 f32); each chunk's x is
    DMA'd as 3 separate k-piece tiles so layer-1 pair 0 starts as soon
    as the first piece lands and DMA pipelines under compute.
  - Layer 1 as banded matmuls over 128-aligned k-tiles with host-packed
    zero-padded weight blocks (channel-mean folded into PSUM accumulation,
    1/3 folded into W1).
  - Layers 2/3 stay feature-major; layer 3 uses gT as lhsT so the output
    comes out batch-major for a contiguous DMA out.
"""

import sys

sys.path.insert(0, "/opt/trn_rl_repo")

import numpy as np

import concourse.bass as bass
import concourse.mybir as mybir
import concourse.tile as tile
from concourse import bacc
from concourse.bass_utils import run_bass_kernel_spmd

P = 128
N_CORES = 8
B_FULL = 8192
B_SHARD = B_FULL // N_CORES          # 1024
CH_LEN = 2048
N_CH = 3
K_FULL = N_CH * CH_LEN               # 6144
N_WIN = 16
WIN = 256
STRIDE = 119
N_PAIR = 8                           # window pairs (2 windows x 64 = 128 feats)
KT_CH = CH_LEN // P                  # 16 k-tiles per channel
KT_ALL = K_FULL // P                 # 48
NB = 512                             # max batch chunk (matmul free dim)
CHUNKS = [512, 512]                  # batch chunk sizes (sum = B_SHARD)
assert sum(CHUNKS) == B_SHARD
assert all(nb % P == 0 for nb in CHUNKS)
# k-split of each chunk's DMA into separate piece tiles, matched to the pair
# bands (m0:0-2, m1:1-4, m2:3-6, m3:5-8, m4:7-10, m5:9-12, m6:11-14,
# m7:13-15) so each pair unblocks as early as possible.
K_PIECES = [(0, 3), (3, 5), (5, 7), (7, 9), (9, 11), (11, 13), (13, 16)]
KP_MAX = max(t1 - t0 for t0, t1 in K_PIECES)


def _piece_of(t):
    for pi, (t0, t1) in enumerate(K_PIECES):
        if t0 <= t < t1:
            return pi, t - t0
    raise ValueError(t)
N_OUT = 255

def _pair_tiles(m):
    """k-tiles of one channel that intersect window pair m (rows 238m..238m+374)."""
    lo = (2 * STRIDE * m) // P
    hi = (2 * STRIDE * m + 2 * STRIDE + WIN - 2 - STRIDE) // P  # (238m+374)//128
    return list(range(lo, min(hi, KT_CH - 1) + 1))

# Block order for layer-1 packed weights: for m, for c, for t.
BLOCKS = [(m, c, t) for m in range(N_PAIR) for c in range(N_CH) for t in _pair_tiles(m)]
BLK_IDX = {key: i for i, key in enumerate(BLOCKS)}
N_BLK = len(BLOCKS)                  # 90


def _pack_weights(W1, b1, W2, b2, W3, b3):
    """Host-side packing of the tiny weight tensors into device layouts.

    If W1_F8: W1 is scaled by W1_SCALE and stored e3m4 (layer-1 outputs come
    out scaled; 1/W1_SCALE is folded into W2, and W1_SCALE into b1).
    """
    W1 = np.asarray(W1, dtype=np.float32)
    ki = np.arange(P)[:, None]                    # tile-local k row
    j = np.arange(P)[None, :]                     # pair-local output feature
    w_off = j // 64                               # window within pair
    n = j % 64

    w1p = np.zeros((N_BLK, P, P), dtype=np.float32)
    for i, (m, c, t) in enumerate(BLOCKS):
        w = 2 * m + w_off                         # [1,128] window index
        koff = P * t + ki - STRIDE * w            # [128,128] k within window
        mask = (koff >= 0) & (koff < WIN)
        w1p[i] = np.where(
            mask, W1[c, w, np.clip(koff, 0, WIN - 1), n] / 3.0, 0.0
        )
    # device layout: [P(ki), N_BLK * P(j)] contiguous per partition
    w1flat = np.ascontiguousarray(w1p.transpose(1, 0, 2).reshape(P, N_BLK * P))
    if W1_F8:
        import ml_dtypes
        w1sb = (w1flat * W1_SCALE).astype(ml_dtypes.float8_e3m4)
    else:
        w1sb = w1flat.astype(np.float16)

    if FOLD_W23:
        # W23_g = W2_g @ W3[64g:64g+64]  [4][256,255]; split k into 2 tiles
        # of 128 and j into halves [128,127]; layout [P, (g kt jh), <=128]
        W2f = np.asarray(W2, dtype=np.float32)
        W3f = np.asarray(W3, dtype=np.float32)
        w23 = np.stack(
            [W2f[g] @ W3f[64 * g:64 * g + 64, :] for g in range(4)]
        )                                          # [4, 256, 255]
        if W1_F8:
            w23 = w23 / W1_SCALE
        w23 = w23.reshape(4, 2, P, 255)            # [g, kt, 128, 255]
        w23p = np.zeros((4, 2, 2, P, P), dtype=np.float32)
        w23p[:, :, 0, :, :] = w23[:, :, :, :128]
        w23p[:, :, 1, :, :127] = w23[:, :, :, 128:]
        w2sb = np.ascontiguousarray(
            w23p.transpose(3, 0, 1, 2, 4).reshape(P, 16 * P)
        ).astype(np.float16)
    else:
        # W2 [4,256,64] -> pieces [g,p][128,64] -> [P, 8, 64]
        w2p = np.asarray(W2, dtype=np.float32).reshape(4, 2, P, 64)
        if W1_F8:
            w2p = w2p / W1_SCALE
        w2sb = np.ascontiguousarray(
            w2p.transpose(2, 0, 1, 3).reshape(P, 8 * 64)
        ).astype(np.float16)

    # W3 [256,255] -> [P, 2, 255]
    w3p = np.asarray(W3, dtype=np.float32).reshape(2, P, N_OUT)
    w3sb = np.ascontiguousarray(
        w3p.transpose(1, 0, 2).reshape(P, 2 * N_OUT)
    ).astype(np.float16)

    # biases (per-partition layouts)
    b1m = np.asarray(b1, dtype=np.float32).mean(axis=0)        # [16,64]
    if W1_F8:
        b1m = b1m * W1_SCALE
    b1t = np.ascontiguousarray(b1m.reshape(N_PAIR, P).T)       # [128, 8]
    b2t = np.ascontiguousarray(np.asarray(b2, dtype=np.float32).T)  # [64, 4]
    if FOLD_W23:
        # b2 flows through W3; b3 applied feature-major [255, 1]
        b3e = np.asarray(b3, np.float32) + (
            np.asarray(b2, np.float32).reshape(256) @ np.asarray(W3, np.float32)
        )
        b3t = np.ascontiguousarray(b3e[:, None])               # [255, 1]
    else:
        b3t = np.ascontiguousarray(
            np.broadcast_to(np.asarray(b3, dtype=np.float32), (P, N_OUT))
        )                                                      # [128, 255]
    return w1sb, w2sb, w3sb, b1t, b2t, b3t


def _pack_x_shard(x16_shard):
    """[1024, 6144] fp16 -> chunk-major feature-major [128, 48*1024].

    For each batch chunk ch (nb rows starting at b0), the block
    [128 partitions, 48*nb] holds xT[k, b] with k = 128*t + p, laid out
    t-major then b within the chunk; chunks are concatenated along the
    free axis so each chunk is one fully contiguous DMA.
    """
    xT = np.ascontiguousarray(x16_shard.T)        # [6144, 1024]
    parts = []
    b0 = 0
    for nb in CHUNKS:
        blk = xT[:, b0:b0 + nb].reshape(KT_ALL, P, nb).transpose(1, 0, 2)
        parts.append(blk.reshape(P, KT_ALL * nb))
        b0 += nb
    return np.ascontiguousarray(np.concatenate(parts, axis=1))


def build_kernel(reps=1, has_bias=False, mode="full", x_f8=None):
    if x_f8 is None:
        x_f8 = X_F8
    nc = bacc.Bacc("TRN2", target_bir_lowering=False, debug=False,
                   num_devices=N_CORES)
    f16 = mybir.dt.float16
    f32 = mybir.dt.float32
    xdt = mybir.dt.float8e3 if x_f8 else f16

    wdt = mybir.dt.float8e3 if W1_F8 else f16
    x_ext = nc.declare_dram_parameter("x", [P, KT_ALL * B_SHARD], xdt, isOutput=False)
    w1_ext = nc.declare_dram_parameter("w1", [P, N_BLK * P], wdt, isOutput=False)
    w2_ext = nc.declare_dram_parameter(
        "w2", [P, (16 * P) if FOLD_W23 else (8 * 64)], f16, isOutput=False)
    w3_ext = nc.declare_dram_parameter("w3", [P, 2 * N_OUT], f16, isOutput=False)
    b1_ext = nc.declare_dram_parameter("b1t", [P, N_PAIR], f32, isOutput=False)
    b2_ext = nc.declare_dram_parameter("b2t", [64, 4], f32, isOutput=False)
    b3_ext = nc.declare_dram_parameter(
        "b3t", [N_OUT, 1] if FOLD_W23 else [P, N_OUT], f32, isOutput=False)
    out_ext = nc.declare_dram_parameter(
        "out", [N_OUT, B_SHARD] if FOLD_W23 else [B_SHARD, N_OUT], f32,
        isOutput=True)

    with tile.TileContext(nc) as tc:
        with (
            tc.tile_pool(name="wpool", bufs=1) as wpool,
            tc.tile_pool(name="xt", bufs=10) as xt_pool,
            tc.tile_pool(name="hp", bufs=10) as hp_pool,
            tc.tile_pool(name="gt", bufs=2) as gt_pool,
            tc.tile_pool(name="osb", bufs=2) as out_pool,
            tc.tile_pool(name="ps1", bufs=4, space="PSUM") as ps1_pool,
            tc.tile_pool(name="ps2", bufs=2, space="PSUM") as ps2_pool,
            tc.tile_pool(name="ps3", bufs=2, space="PSUM") as ps3_pool,
        ):
            w1sb = wpool.tile([P, N_BLK, P], wdt)
            nc.scalar.dma_start(out=w1sb[:], in_=w1_ext.rearrange("p (b j) -> p b j", j=P))
            if FOLD_W23:
                w2sb = wpool.tile([P, 16, P], f16)
                nc.scalar.dma_start(
                    out=w2sb[:], in_=w2_ext.rearrange("p (b j) -> p b j", j=P))
            else:
                w2sb = wpool.tile([P, 8, 64], f16)
                nc.scalar.dma_start(
                    out=w2sb[:], in_=w2_ext.rearrange("p (b j) -> p b j", j=64))
            w3sb = wpool.tile([P, 2, N_OUT], f16)
            nc.scalar.dma_start(out=w3sb[:], in_=w3_ext.rearrange("p (b j) -> p b j", j=N_OUT))
            b1sb = wpool.tile([P, N_PAIR], f32)
            nc.scalar.dma_start(out=b1sb[:], in_=b1_ext[:])
            b2sb = wpool.tile([64, 4], f32)
            nc.scalar.dma_start(out=b2sb[:], in_=b2_ext[:])
            b3sb = wpool.tile([N_OUT, 1] if FOLD_W23 else [P, N_OUT], f32)
            nc.scalar.dma_start(out=b3sb[:], in_=b3_ext[:])

            xt_fix = None
            if mode == "compute":
                # persistent x chunk for compute-only probe; also satisfy out
                xt_fix = wpool.tile([P, KT_ALL, NB], xdt)
                nc.sync.dma_start(
                    out=xt_fix[:, :, :],
                    in_=x_ext[:, :KT_ALL * NB].rearrange("p (t j) -> p t j", j=NB),
                )
            if mode == "dma":
                # out is never written in the loop; write something once
                nc.scalar.dma_start(out=out_ext[0:P, :], in_=b3sb[:])

            import contextlib
            loop_cm = tc.For_i(0, reps, 1) if reps > 1 else contextlib.nullcontext()
            with loop_cm:
                _kernel_body(nc, tc, locals(), has_bias, mode, xdt, xt_fix)

    nc.compile()
    return nc


def _kernel_body(nc, tc, env, has_bias, mode="full", xdt=None, xt_fix=None):
    x_ext = env["x_ext"]
    out_ext = env["out_ext"]
    w1sb, w2sb, w3sb = env["w1sb"], env["w2sb"], env["w3sb"]
    b1sb, b2sb, b3sb = env["b1sb"], env["b2sb"], env["b3sb"]
    xt_pool = env["xt_pool"]
    hp_pool, gt_pool, out_pool = env["hp_pool"], env["gt_pool"], env["out_pool"]
    ps1_pool, ps2_pool, ps3_pool = env["ps1_pool"], env["ps2_pool"], env["ps3_pool"]
    f16 = mybir.dt.float16
    f32 = mybir.dt.float32
    if xdt is None:
        xdt = f16

    b0 = 0
    for ch, nb in enumerate(CHUNKS):
        if mode == "compute":
            xtv = xt_fix[:, :, :].rearrange("p (c t) j -> p c t j", c=N_CH)
            pieces = None
        else:
            # chunk ch of pre-transposed x, one separate tile per k-piece so
            # layer-1 pairs start as soon as their piece lands
            c0 = KT_ALL * b0
            src = x_ext[:, c0:c0 + KT_ALL * nb].rearrange(
                "p (c t j) -> p c t j", c=N_CH, j=nb
            )
            pieces = []
            for (t0, t1) in K_PIECES:
                xp_t = xt_pool.tile([P, N_CH, KP_MAX, NB], xdt, name="xpt")
                xp = xp_t[:, :, :t1 - t0, :nb]
                nc.sync.dma_start(out=xp[:], in_=src[:, :, t0:t1, :])
                pieces.append(xp)
        if mode == "dma":
            b0 += nb
            continue

        def xt_rhs(c, t):
            if pieces is None:
                return xtv[:, c, t, :]
            pi, tl = _piece_of(t)
            return pieces[pi][:, c, tl, :]

        # ---- layer 1: banded matmuls per window pair ----
        hps = {}
        for m in range(N_PAIR):
            ps_t = ps1_pool.tile([P, NB], f32, name="ps1t")
            ps = ps_t[:, :nb]
            mm_list = [(c, t) for c in range(N_CH) for t in _pair_tiles(m)]
            for i, (c, t) in enumerate(mm_list):
                nc.tensor.matmul(
                    ps[:],
                    w1sb[:, BLK_IDX[(m, c, t)], :],
                    xt_rhs(c, t),
                    start=(i == 0),
                    stop=(i == len(mm_list) - 1),
                )
            hp_t = hp_pool.tile([P, NB], f16, name="hpt")
            hp = hp_t[:, :nb]
            if has_bias:
                nc.vector.tensor_scalar_add(hp[:], ps[:], b1sb[:, m:m + 1])
            else:
                nc.vector.tensor_copy(out=hp[:], in_=ps[:])
            hps[m] = hp

        if FOLD_W23:
            # ---- fused layers 2+3: out_jh = sum_(g,kt) h @ W23 ----
            for jh in range(2):
                jw = P if jh == 0 else N_OUT - P
                psO_t = ps2_pool.tile([P, NB], f32, name="psot")
                psO = psO_t[:jw, :nb]
                for i in range(8):
                    g, kt = i // 2, i % 2
                    nc.tensor.matmul(
                        psO[:],
                        w2sb[:, (g * 2 + kt) * 2 + jh, :jw],
                        hps[2 * g + kt][:],
                        start=(i == 0),
                        stop=(i == 7),
                    )
                osb_t = out_pool.tile([P, NB], f32, name="osbt")
                osb = osb_t[:jw, :nb]
                if has_bias:
                    nc.vector.tensor_scalar_add(
                        osb[:], psO[:], b3sb[jh * P:jh * P + jw, 0:1])
                else:
                    nc.vector.tensor_copy(out=osb[:], in_=psO[:])
                nc.scalar.dma_start(
                    out=out_ext[jh * P:jh * P + jw, b0:b0 + nb], in_=osb[:],
                )
            b0 += nb
            continue

        # ---- layer 2: 4 groups of 4 windows ----
        gt_t = gt_pool.tile([P, 2, NB], f16, name="gtt")
        gt = gt_t[:, :, :nb]
        for g in range(4):
            ps2_t = ps2_pool.tile([64, NB], f32, name="ps2t")
            ps2 = ps2_t[:, :nb]
            for piece in range(2):
                nc.tensor.matmul(
                    ps2[:],
                    w2sb[:, 2 * g + piece, :],
                    hps[2 * g + piece][:],
                    start=(piece == 0),
                    stop=(piece == 1),
                )
            lo = 64 * (g % 2)
            if has_bias:
                nc.vector.tensor_scalar_add(
                    gt[lo:lo + 64, g // 2], ps2[:], b2sb[:, g:g + 1],
                )
            else:
                nc.vector.tensor_copy(out=gt[lo:lo + 64, g // 2], in_=ps2[:])

        # ---- layer 3: back to batch-major ----
        nj = nb // P
        osb_t = out_pool.tile([P, NB // P, N_OUT], f32, name="osbt")
        osb = osb_t[:, :nj]
        for js in range(nj):
            ps3 = ps3_pool.tile([P, N_OUT], f32)
            for piece in range(2):
                nc.tensor.matmul(
                    ps3[:],
                    gt[:, piece, js * P:(js + 1) * P],
                    w3sb[:, piece, :],
                    start=(piece == 0),
                    stop=(piece == 1),
                )
            if has_bias:
                nc.vector.tensor_tensor(
                    osb[:, js], ps3[:], b3sb[:], mybir.AluOpType.add,
                )
            else:
                nc.vector.tensor_copy(out=osb[:, js], in_=ps3[:])
        nc.scalar.dma_start(
            out=out_ext[b0:b0 + nb, :].rearrange("(j p) n -> p j n", p=P),
            in_=osb[:],
        )
        b0 += nb


_CACHED_NC = None

# Ship x as fp8 e3m4: halves HBM traffic but adds a per-matmul rhs-stream
# penalty that outweighs it (measured 72.6us vs 67.2us fp16, and rel err
# rises to 1.90e-2 vs 1.35e-2). Keep fp16 x.
X_F8 = False

# Store W1 as fp8 e3m4 (scaled by W1_SCALE into e3m4's dense range): cuts
# LDWEIGHTS stream time 2x via 4-byte FWL. 1/W1_SCALE folds into W2.
W1_F8 = True
W1_SCALE = 64.0

# Fold W3 into W2 host-side: out = sum_g h_g @ (W2_g @ W3_blk). Halves the
# layer-2/3 matmul count and removes the gt copies; output leaves the device
# feature-major [255, B_SHARD] and is transposed on the host.
FOLD_W23 = False


def _prep_in_maps(x, W1, b1, W2, b2, W3, b3):
    import ml_dtypes
    xdt = ml_dtypes.float8_e3m4 if X_F8 else np.float16
    xcast = np.asarray(x, dtype=np.float32).astype(xdt)
    w1sb, w2sb, w3sb, b1t, b2t, b3t = _pack_weights(W1, b1, W2, b2, W3, b3)
    in_maps = []
    for i in range(N_CORES):
        in_maps.append({
            "x": _pack_x_shard(xcast[i * B_SHARD:(i + 1) * B_SHARD]),
            "w1": w1sb,
            "w2": w2sb,
            "w3": w3sb,
            "b1t": b1t,
            "b2t": b2t,
            "b3t": b3t,
        })
    return in_maps


_CACHED_BIAS_NC = None


def kernel(x, W1, b1, W2, b2, W3, b3):
    global _CACHED_NC, _CACHED_BIAS_NC
    has_bias = bool(
        np.any(np.asarray(b1)) or np.any(np.asarray(b2)) or np.any(np.asarray(b3))
    )
    if has_bias:
        if _CACHED_BIAS_NC is None:
            _CACHED_BIAS_NC = build_kernel(has_bias=True, x_f8=X_F8)
        nc = _CACHED_BIAS_NC
    else:
        if _CACHED_NC is None:
            _CACHED_NC = build_kernel(x_f8=X_F8)
        nc = _CACHED_NC
    in_maps = _prep_in_maps(x, W1, b1, W2, b2, W3, b3)
    last_err = None
    for attempt in range(3):
        try:
            res = run_bass_kernel_spmd(nc, in_maps, core_ids=list(range(N_CORES)))
            break
        except Exception as e:  # transient device/axon failures
            last_err = e
            if attempt == 2:
                raise
            import time as _time
            _time.sleep(20.0)
    if FOLD_W23:
        return np.concatenate(
            [res.results[i]["out"].T for i in range(N_CORES)], axis=0)
    return np.concatenate([res.results[i]["out"] for i in range(N_CORES)], axis=0)
